# revision 1
# baseline (speedup 1.0000x reference)
"""Two-layer GAT (PyG GATConv semantics) on 8 TRN2 NeuronCores.

Strategy (edge/graph parallel):
  - Host (index manipulation only): sort non-self-loop edges by dst, shard
    dst nodes contiguously across the 8 cores, pad each dst tile's edge
    list to a multiple of 128, and precompute each 128-edge tile's 0/1
    incidence matrices S (dst x edge) and S^T as bf16 (pure index data).
  - Phase A (per core, redundant): HAUG[v] = [x@W1 + b1 | a_src | a_dst]
    (272 cols, bf16) via one fused matmul -- the per-head attention
    reductions are folded into the weight matrix by associativity:
    (x@W1).a = x@(W1.a_blockdiag).
  - Phase B (edge phase, layer 1): per dst tile accumulate in PSUM over
    128-edge tiles: indirect-gather HAUG[src]; alpha_dst per edge via a
    small matmul S.T @ a_dst_tile (a_dst_tile is a direct contiguous
    load); exp(leakyrelu(asrc+adst)) per edge/head; one incidence matmul
    aggregates [sum exp*h | sum exp].  Self-loop contributions are added
    densely in the flush (no gather).  Normalize (softmax denominator
    folded in), ELU, store z (bf16).
  - Phase C: T2[v] = [z@W2 + b2 | a2_src | a2_dst] per-core shard; one
    AllGather.
  - Phase D: layer-2 edge phase (heads=1) -> output shard.

All floating-point math happens on device; the host only reorders
indices and pads/reshapes layouts.
"""

import contextlib

import numpy as np

import concourse.bass as bass
import concourse.bacc as bacc
import concourse.mybir as mybir
import concourse.tile as tile
from concourse.bass_utils import run_bass_kernel_spmd

# ---- fixed problem hyperparameters (from the nn.Module) ----
F_IN = 256
H = 8
C = 32
NCLS = 40
NEG = 0.2

W = 8               # cores
P = 128             # partitions
D1 = F_IN + 2 * H   # HAUG row: [h+b1 (256) | a_src (8) | a_dst (8)] = 272
D2 = NCLS + 8       # T2 row: [h2+b2 (40) | a2_src (1) | a2_dst x7 (41:48)]
R = 4               # edge tiles batched per compute group

f32 = mybir.dt.float32
bf16 = mybir.dt.bfloat16
i32 = mybir.dt.int32
BF_NP = mybir.dt.np(bf16)

Exp = mybir.ActivationFunctionType.Exp
ADD = mybir.AluOpType.add
MULT = mybir.AluOpType.mult
MAX = mybir.AluOpType.max

TRACE = False       # set by test harness for profiling runs
_CACHE = {}


def _host_prep(x, edge_index):
    """Index-only preprocessing. Returns (meta, per-core arrays)."""
    N = x.shape[0]
    E = edge_index.shape[1]
    # NOTE: keep edge_index as-is (including any pre-existing (v, v)
    # edges); only the appended PyG self-loop is handled densely in the
    # flush.
    src_a = np.asarray(edge_index[0], np.int64)
    dst_a = np.asarray(edge_index[1], np.int64)
    order = np.argsort(dst_a, kind="stable")
    src_s = src_a[order].astype(np.int32)
    dst_s = dst_a[order].astype(np.int32)

    nt_real = -(-N // P)
    T = -(-nt_real // W)
    SH = T * P
    NPAD = W * SH

    gt = dst_s // P
    cnt = np.bincount(gt, minlength=W * T)
    K = np.maximum(1, -(-cnt.reshape(W, T).max(axis=0) // P))
    offs = np.zeros(T + 1, np.int64)
    offs[1:] = np.cumsum(K)
    CT = int(offs[-1])

    start_gt = np.zeros(W * T + 1, np.int64)
    start_gt[1:] = np.cumsum(cnt)
    q = np.arange(len(dst_s)) - start_gt[gt]
    c_all = gt // T
    t_all = gt % T
    col = offs[t_all] + q // P
    p_all = q % P

    src_arr = np.zeros((W, P, CT), np.int32)
    dstl_arr = np.full((W, P, CT), -1, np.int16)
    src_arr[c_all, p_all, col] = src_s
    dstl_arr[c_all, p_all, col] = (dst_s % P).astype(np.int16)

    meta = dict(N=N, E=E, T=T, SH=SH, NPAD=NPAD, K=tuple(int(k) for k in K),
                offs=offs, CT=CT)

    xT = np.zeros((F_IN, NPAD), np.float32)
    xT[:, :N] = np.asarray(x, np.float32).T
    xT = xT.astype(BF_NP)

    # Layer-1 table is rotated per core so each core's own dst shard sits
    # at rows [0, SH) of its private HAUG (the program is identical across
    # cores; per-core data encodes the shard).  srcg is remapped to match.
    xT_rot = [np.roll(xT, -c * SH, axis=1) for c in range(W)]
    srcg_rot = [(src_arr[c] + (NPAD - c * SH)) % NPAD for c in range(W)]

    arrays = dict(src_arr=src_arr, dstl_arr=dstl_arr, xT_rot=xT_rot,
                  srcg_rot=srcg_rot)
    return meta, arrays


def _make_sst(dstl_core):
    """[P, CT] int16 dst-local (-1 pad) -> [CT, P, 256] bf16 [S | S^T]."""
    CT = dstl_core.shape[1]
    d = dstl_core.T                          # [CT, 128] values in [-1, 128)
    iota = np.arange(P, dtype=np.int16)
    S = (d[:, None, :] == iota[None, :, None])       # [CT, d, e]
    out = np.empty((CT, P, 2 * P), BF_NP)
    out[:, :, :P] = S.astype(BF_NP)
    out[:, :, P:] = S.transpose(0, 2, 1).astype(BF_NP)
    return out


def _edge_phase(nc, tc, ctx, meta, consts, *, table, srcb, self_ap, dglen,
                nheads, hw, flush_fn, tag):
    """Edge phase shared by both layers.

    table: DRAM gather table [NPAD, DT]; row = [h (dglen) | a_src (nheads)
    | a_dst ...].  alpha_dst slice of a row starts at dglen + nheads.
    flush_fn(t, agg, selfrow, fpool): consume accumulated PSUM
    [P, dglen+nheads] plus the dst tile's own table rows.
    """
    T, K, offs = meta["T"], meta["K"], meta["offs"]
    DT = table.shape[1]
    DA = dglen + nheads              # aggregation width: [scaled h | exp]
    sst_d = consts["sst_d"]

    pool = consts["pool"]
    spool = consts["spool"]
    fpool = consts["fpool"]
    psum = consts["psum"]
    apsum = consts["apsum"]

    for t in range(T):
        agg = psum.tile([P, DA], f32, tag="agg")
        nj = K[t]
        base = int(offs[t])
        # dst tile's own table rows: alpha_dst source + self-loop terms
        selfrow = fpool.tile([P, DT], bf16, tag="selfrow")
        nc.sync.dma_start(out=selfrow[:], in_=self_ap(t))
        for j0 in range(0, nj, R):
            r = min(R, nj - j0)
            g = pool.tile([P, R, DT], bf16, tag="g")
            for ri in range(r):
                c1 = base + j0 + ri
                nc.gpsimd.indirect_dma_start(
                    out=g[:, ri, :], out_offset=None, in_=table[:],
                    in_offset=bass.IndirectOffsetOnAxis(ap=srcb[:, c1:c1 + 1], axis=0),
                )
            sst = spool.tile([P, R, 2 * P], bf16, tag="sst")
            nc.scalar.dma_start(
                out=sst[:, :r, :],
                in_=sst_d[base + j0:base + j0 + r, :, :].rearrange("c p d -> p c d"))
            ade = apsum.tile([P, R, nheads], f32, tag="ade")
            for ri in range(r):
                nc.tensor.matmul(
                    ade[:, ri, :], lhsT=sst[:, ri, :P],
                    rhs=selfrow[:, dglen + nheads:dglen + 2 * nheads],
                    start=True, stop=True)
            s = pool.tile([P, R, nheads], f32, tag="s")
            nc.vector.tensor_add(out=s[:, :r], in0=g[:, :r, dglen:dglen + nheads],
                                 in1=ade[:, :r])
            e = pool.tile([P, R, nheads], f32, tag="e")
            nc.vector.scalar_tensor_tensor(
                out=e[:, :r], in0=s[:, :r], scalar=NEG, in1=s[:, :r],
                op0=MULT, op1=MAX)
            rhs = pool.tile([P, R, DA], bf16, tag="rhs")
            nc.scalar.activation(out=rhs[:, :r, dglen:], in_=e[:, :r], func=Exp)
            nc.vector.tensor_tensor(
                out=rhs[:, :r, :dglen].rearrange("p r (h c) -> p r h c", h=nheads),
                in0=g[:, :r, :dglen].rearrange("p r (h c) -> p r h c", h=nheads),
                in1=rhs[:, :r, dglen:].rearrange("p r (h o) -> p r h o", o=1)
                    .to_broadcast([P, r, nheads, hw]),
                op=MULT)
            for ri in range(r):
                nc.tensor.matmul(
                    agg[:], lhsT=sst[:, ri, P:], rhs=rhs[:, ri, :],
                    start=(j0 == 0 and ri == 0), stop=(j0 + ri == nj - 1))
        flush_fn(t, agg, selfrow, fpool)


def _build_program(meta):
    T, SH, NPAD, CT = meta["T"], meta["SH"], meta["NPAD"], meta["CT"]

    nc = bacc.Bacc("TRN2", target_bir_lowering=False, debug=False, num_devices=W)

    xT_d = nc.dram_tensor("xT", [F_IN, NPAD], bf16, kind="ExternalInput")
    w1_d = nc.dram_tensor("W1", [F_IN, F_IN], f32, kind="ExternalInput")
    asrc_d = nc.dram_tensor("asrc", [1, F_IN], f32, kind="ExternalInput")
    adstv_d = nc.dram_tensor("adstv", [1, F_IN], f32, kind="ExternalInput")
    b1_d = nc.dram_tensor("b1", [1, F_IN], f32, kind="ExternalInput")
    w2_d = nc.dram_tensor("W2", [F_IN, NCLS], f32, kind="ExternalInput")
    a2s_d = nc.dram_tensor("a2s", [1, NCLS], f32, kind="ExternalInput")
    a2d_d = nc.dram_tensor("a2d", [1, NCLS], f32, kind="ExternalInput")
    b2_d = nc.dram_tensor("b2", [1, NCLS], f32, kind="ExternalInput")
    srcg_d = nc.dram_tensor("srcg", [P, CT], i32, kind="ExternalInput")
    srcg2_d = nc.dram_tensor("srcg2", [P, CT], i32, kind="ExternalInput")
    sst_d = nc.dram_tensor("sst", [CT, P, 2 * P], bf16, kind="ExternalInput")
    out_d = nc.dram_tensor("out", [SH, NCLS], f32, kind="ExternalOutput")

    HAUG = nc.dram_tensor("HAUG", [NPAD, D1], bf16)
    Z = nc.dram_tensor("Z", [SH, F_IN], bf16)

    with tile.TileContext(nc) as tc:
        with contextlib.ExitStack() as top:
            cpool = top.enter_context(tc.tile_pool(name="const", bufs=1))
            dram = top.enter_context(tc.tile_pool(name="dram", bufs=1, space="DRAM"))

            srcb = cpool.tile([P, CT], i32)
            nc.sync.dma_start(out=srcb[:], in_=srcg_d[:])
            srcb2 = cpool.tile([P, CT], i32)
            nc.sync.dma_start(out=srcb2[:], in_=srcg2_d[:])

            rhs1 = [cpool.tile([P, D1], bf16, name=f"rhs1_{k}") for k in range(2)]
            rhs2 = [cpool.tile([P, D2], bf16, name=f"rhs2_{k}") for k in range(2)]
            b1_b = cpool.tile([P, F_IN], f32)
            b1e_b = cpool.tile([P, D1], f32)
            b2p_b = cpool.tile([P, D2], f32)

            # ---- setup: broadcast rows + fold attention vectors into rhs ----
            with contextlib.ExitStack() as su:
                spool = su.enter_context(tc.tile_pool(name="setup", bufs=1))
                spsum = su.enter_context(tc.tile_pool(name="setup_ps", bufs=1, space="PSUM"))
                ones = spool.tile([1, P], f32)
                nc.vector.memset(ones[:], 1.0)

                def bcast(dram_ap, width, out_ap):
                    ps = spsum.tile([P, width], f32, tag="bps")
                    row = spool.tile([1, width], f32, tag="brow")
                    nc.sync.dma_start(out=row[:], in_=dram_ap)
                    nc.tensor.matmul(ps[:], lhsT=ones[:], rhs=row[:], start=True, stop=True)
                    nc.vector.tensor_copy(out=out_ap, in_=ps[:])

                asrc_b = spool.tile([P, F_IN], f32)
                bcast(asrc_d[:], F_IN, asrc_b[:])
                adst_b = spool.tile([P, F_IN], f32)
                bcast(adstv_d[:], F_IN, adst_b[:])
                bcast(b1_d[:], F_IN, b1_b[:])
                nc.vector.memset(b1e_b[:], 0.0)
                nc.vector.tensor_copy(out=b1e_b[:, :F_IN], in_=b1_b[:])
                a2s_b = spool.tile([P, NCLS], f32)
                bcast(a2s_d[:], NCLS, a2s_b[:])
                a2d_b = spool.tile([P, NCLS], f32)
                bcast(a2d_d[:], NCLS, a2d_b[:])
                nc.vector.memset(b2p_b[:], 0.0)
                bcast(b2_d[:], NCLS, b2p_b[:, :NCLS])

                for k in range(2):
                    w1sb = spool.tile([P, F_IN], f32, tag="w1sb")
                    nc.sync.dma_start(out=w1sb[:], in_=w1_d[k * P:(k + 1) * P, :])
                    nc.vector.tensor_copy(out=rhs1[k][:, :F_IN], in_=w1sb[:])
                    for vec_b, col in ((asrc_b, F_IN), (adst_b, F_IN + H)):
                        tmp = spool.tile([P, F_IN], f32, tag="tmp")
                        nc.vector.tensor_mul(out=tmp[:], in0=w1sb[:], in1=vec_b[:])
                        vred = spool.tile([P, H], f32, tag="vred")
                        nc.vector.tensor_reduce(
                            out=vred[:], in_=tmp[:].rearrange("p (h c) -> p h c", h=H),
                            axis=mybir.AxisListType.X, op=ADD)
                        nc.vector.tensor_copy(out=rhs1[k][:, col:col + H], in_=vred[:])

                    w2sb = spool.tile([P, NCLS], f32, tag="w2sb")
                    nc.sync.dma_start(out=w2sb[:], in_=w2_d[k * P:(k + 1) * P, :])
                    nc.vector.tensor_copy(out=rhs2[k][:, :NCLS], in_=w2sb[:])
                    for vec_b, cs in ((a2s_b, slice(NCLS, NCLS + 1)),
                                      (a2d_b, slice(NCLS + 1, D2))):
                        tmp2 = spool.tile([P, NCLS], f32, tag="tmp2")
                        nc.vector.tensor_mul(out=tmp2[:], in0=w2sb[:], in1=vec_b[:])
                        vred2 = spool.tile([P, 1], f32, tag="vred2")
                        nc.vector.tensor_reduce(
                            out=vred2[:], in_=tmp2[:].rearrange("p (o c) -> p o c", o=1),
                            axis=mybir.AxisListType.X, op=ADD)
                        n_rep = cs.stop - cs.start
                        nc.vector.tensor_copy(
                            out=rhs2[k][:, cs], in_=vred2[:].to_broadcast([P, n_rep]))

            # ---- Phase A: HAUG for all NPAD nodes (redundant per core) ----
            with contextlib.ExitStack() as pa:
                apool = pa.enter_context(tc.tile_pool(name="pa", bufs=4))
                apsum = pa.enter_context(tc.tile_pool(name="pa_ps", bufs=2, space="PSUM"))
                NB = 1024
                for b in range(NPAD // NB):
                    xt = [apool.tile([P, NB], bf16, tag=f"xt{k}", name=f"xt{k}")
                          for k in range(2)]
                    for k in range(2):
                        nc.sync.dma_start(
                            out=xt[k][:], in_=xT_d[k * P:(k + 1) * P, b * NB:(b + 1) * NB])
                    hsb = apool.tile([P, NB // P, D1], bf16, tag="hsb")
                    for nt in range(NB // P):
                        ps = apsum.tile([P, D1], f32, tag="aps")
                        for k in range(2):
                            nc.tensor.matmul(
                                ps[:], lhsT=xt[k][:, nt * P:(nt + 1) * P], rhs=rhs1[k][:],
                                start=(k == 0), stop=(k == 1))
                        nc.vector.tensor_add(out=hsb[:, nt, :], in0=ps[:],
                                             in1=b1e_b[:])
                    row0 = b * NB
                    nc.scalar.dma_start(
                        out=HAUG[row0:row0 + NB, :].rearrange("(a p) d -> p a d", p=P),
                        in_=hsb[:])

            cpool2 = top.enter_context(tc.tile_pool(name="pc", bufs=3))
            cpsum = top.enter_context(tc.tile_pool(name="pc_ps", bufs=2, space="PSUM"))
            T2L = dram.tile([SH, D2], bf16, name="T2L")
            ep_pool = top.enter_context(tc.tile_pool(name="ep", bufs=8))
            ep_spool = top.enter_context(tc.tile_pool(name="ep_s", bufs=6))
            ep_fpool = top.enter_context(tc.tile_pool(name="ep_f", bufs=3))
            ep_psum = top.enter_context(tc.tile_pool(name="ep_p", bufs=3, space="PSUM"))
            ep_apsum = top.enter_context(tc.tile_pool(name="ep_a", bufs=3, space="PSUM"))
            consts = dict(sst_d=sst_d, pool=ep_pool, spool=ep_spool,
                          fpool=ep_fpool, psum=ep_psum, apsum=ep_apsum)

            # ---- Phase B: layer-1 edge phase -> Z ----
            def flush1(t, agg, selfrow, fpool):
                # self-loop terms (dense)
                es = fpool.tile([P, H], f32, tag="es")
                nc.vector.tensor_add(out=es[:], in0=selfrow[:, F_IN:F_IN + H],
                                     in1=selfrow[:, F_IN + H:])
                nc.vector.scalar_tensor_tensor(
                    out=es[:], in0=es[:], scalar=NEG, in1=es[:], op0=MULT, op1=MAX)
                exs = fpool.tile([P, H], f32, tag="exs")
                nc.scalar.activation(out=exs[:], in_=es[:], func=Exp)
                selfsc = fpool.tile([P, F_IN], f32, tag="selfsc")
                nc.vector.tensor_tensor(
                    out=selfsc[:].rearrange("p (h c) -> p h c", h=H),
                    in0=selfrow[:, :F_IN].rearrange("p (h c) -> p h c", h=H),
                    in1=exs[:].rearrange("p (h o) -> p h o", o=1).to_broadcast([P, H, C]),
                    op=MULT)
                numer = fpool.tile([P, F_IN], f32, tag="numer")
                nc.vector.tensor_add(out=numer[:], in0=selfsc[:], in1=agg[:, :F_IN])
                dinv = fpool.tile([P, H], f32, tag="dinv")
                nc.vector.tensor_add(out=dinv[:], in0=exs[:], in1=agg[:, F_IN:])
                nc.vector.tensor_scalar_add(out=dinv[:], in0=dinv[:], scalar1=1e-16)
                nc.vector.reciprocal(out=dinv[:], in_=dinv[:])
                o = fpool.tile([P, F_IN], f32, tag="o")
                nc.vector.tensor_tensor(
                    out=o[:].rearrange("p (h c) -> p h c", h=H),
                    in0=numer[:].rearrange("p (h c) -> p h c", h=H),
                    in1=dinv[:].rearrange("p (h o) -> p h o", o=1).to_broadcast([P, H, C]),
                    op=MULT)
                # ELU -> z (bf16)
                mmin = fpool.tile([P, F_IN], f32, tag="mmin")
                nc.vector.tensor_scalar_min(out=mmin[:], in0=o[:], scalar1=0.0)
                ex = fpool.tile([P, F_IN], f32, tag="ex")
                nc.scalar.activation(out=ex[:], in_=mmin[:], func=Exp)
                rel = fpool.tile([P, F_IN], f32, tag="rel")
                nc.vector.tensor_scalar_max(out=rel[:], in0=o[:], scalar1=0.0)
                z = fpool.tile([P, F_IN], bf16, tag="z")
                nc.vector.scalar_tensor_tensor(
                    out=z[:], in0=ex[:], scalar=-1.0, in1=rel[:], op0=ADD, op1=ADD)
                nc.sync.dma_start(out=Z[t * P:(t + 1) * P, :], in_=z[:])
                # Phase C for this tile, overlapped with the edge phase
                zt = [cpool2.tile([P, P], bf16, tag=f"zt{k}", name=f"zt{k}")
                      for k in range(2)]
                for k in range(2):
                    nc.sync.dma_start(
                        out=zt[k][:], in_=Z[t * P:(t + 1) * P, k * P:(k + 1) * P],
                        transpose=True)
                ps2 = cpsum.tile([P, D2], f32, tag="cps")
                for k in range(2):
                    nc.tensor.matmul(ps2[:], lhsT=zt[k][:], rhs=rhs2[k][:],
                                     start=(k == 0), stop=(k == 1))
                t2sb = cpool2.tile([P, D2], bf16, tag="t2sb")
                nc.vector.tensor_add(out=t2sb[:], in0=ps2[:], in1=b2p_b[:])
                nc.sync.dma_start(out=T2L[t * P:(t + 1) * P, :], in_=t2sb[:])

            with contextlib.ExitStack() as pb:
                _edge_phase(nc, tc, pb, meta, consts, table=HAUG, srcb=srcb,
                            self_ap=lambda t: HAUG[t * P:(t + 1) * P, :],
                            dglen=F_IN, nheads=H, hw=C, flush_fn=flush1, tag="b")

            # ---- AllGather of T2 (Phase C ran inside flush1) ----
            T2F = dram.tile([NPAD, D2], bf16, name="T2F", addr_space="Shared")

            nc.gpsimd.collective_compute(
                "AllGather", mybir.AluOpType.bypass,
                replica_groups=[list(range(W))],
                ins=[T2L.opt()], outs=[T2F.opt()])

            # ---- Phase D: layer-2 edge phase -> out ----
            def flush2(t, agg, selfrow, fpool):
                es = fpool.tile([P, 1], f32, tag="es2")
                nc.vector.tensor_add(out=es[:], in0=selfrow[:, NCLS:NCLS + 1],
                                     in1=selfrow[:, NCLS + 1:NCLS + 2])
                nc.vector.scalar_tensor_tensor(
                    out=es[:], in0=es[:], scalar=NEG, in1=es[:], op0=MULT, op1=MAX)
                exs = fpool.tile([P, 1], f32, tag="exs2")
                nc.scalar.activation(out=exs[:], in_=es[:], func=Exp)
                selfsc = fpool.tile([P, NCLS], f32, tag="selfsc2")
                nc.vector.tensor_tensor(
                    out=selfsc[:], in0=selfrow[:, :NCLS],
                    in1=exs[:].to_broadcast([P, NCLS]), op=MULT)
                numer = fpool.tile([P, NCLS], f32, tag="numer2")
                nc.vector.tensor_add(out=numer[:], in0=selfsc[:], in1=agg[:, :NCLS])
                dinv = fpool.tile([P, 1], f32, tag="dinv2")
                nc.vector.tensor_add(out=dinv[:], in0=exs[:], in1=agg[:, NCLS:])
                nc.vector.tensor_scalar_add(out=dinv[:], in0=dinv[:], scalar1=1e-16)
                nc.vector.reciprocal(out=dinv[:], in_=dinv[:])
                o = fpool.tile([P, NCLS], f32, tag="o2")
                nc.vector.tensor_tensor(
                    out=o[:], in0=numer[:], in1=dinv[:].to_broadcast([P, NCLS]), op=MULT)
                nc.sync.dma_start(out=out_d[t * P:(t + 1) * P, :], in_=o[:])

            with contextlib.ExitStack() as pd:
                _edge_phase(nc, tc, pd, meta, consts, table=T2F.tensor.ap(),
                            srcb=srcb2,
                            self_ap=lambda t: T2L[t * P:(t + 1) * P, :],
                            dglen=NCLS, nheads=1, hw=NCLS, flush_fn=flush2, tag="d")

    nc.compile()
    return nc


def kernel(**inputs):
    x = np.asarray(inputs["x"], np.float32)
    edge_index = np.asarray(inputs["edge_index"])
    meta, arrays = _host_prep(x, edge_index)

    key = (meta["N"], meta["E"], meta["K"])
    if key not in _CACHE:
        _CACHE[key] = _build_program(meta)
    nc = _CACHE[key]

    common = {
        "W1": np.asarray(inputs["W1"], np.float32),
        "asrc": np.asarray(inputs["att_src1"], np.float32).reshape(1, -1),
        "adstv": np.asarray(inputs["att_dst1"], np.float32).reshape(1, -1),
        "b1": np.asarray(inputs["bias1"], np.float32).reshape(1, -1),
        "W2": np.asarray(inputs["W2"], np.float32),
        "a2s": np.asarray(inputs["att_src2"], np.float32).reshape(1, -1),
        "a2d": np.asarray(inputs["att_dst2"], np.float32).reshape(1, -1),
        "b2": np.asarray(inputs["bias2"], np.float32).reshape(1, -1),
    }
    in_maps = []
    for c in range(W):
        m = dict(common)
        m["xT"] = arrays["xT_rot"][c]
        m["srcg"] = arrays["srcg_rot"][c]
        m["srcg2"] = arrays["src_arr"][c]
        m["sst"] = _make_sst(arrays["dstl_arr"][c])
        in_maps.append(m)

    res = run_bass_kernel_spmd(nc, in_maps, core_ids=list(range(W)), trace=TRACE)
    kernel.last_results = res

    N = meta["N"]
    out = np.concatenate([res.results[c]["out"] for c in range(W)], axis=0)
    return np.ascontiguousarray(out[:N])

